# revision 17
# baseline (speedup 1.0000x reference)
"""Single-head attention (b=4, s=4096, d=1024, h=64) on 8 TRN2 NeuronCores.

Sharding: core c handles batch c//2, query half c%2 (2048 queries), with the
full 4096-key context of that batch. No collectives needed. The host
transposes x[b] to x^T [1024, 4096] (bf16) with the core's query columns
rotated to the front, so one SPMD graph serves all 8 cores (softmax is
permutation-invariant over keys).

Pipeline (all matmuls bf16, f32 PSUM):
  Q^T = [Wq|Wq]^T x^T  [128, 2048]: Q duplicated on both partition halves.
  K/V projection in 512-col groups with ALTERNATING packing: even groups
  [Wv|Wk] (K on partitions 64..127), odd groups [Wk|Wv] (K on 0..63).
  Key tiles from an even and an odd group then PAIR: their score matmuls
  (contraction h=64) occupy disjoint PE row groups and run CONCURRENTLY
  (tile_position auto-derived from base partition), halving effective
  score cost on HW. The pairing is invisible to the cost-model sim.
  Per key tile t: S^T[t] = K[t].T Q^T -> PSUM [128, 1024], P^T = exp(S^T)
  -> bf16 (scale-free: 0.125 folded into Wq), O_aug^T += V_aug[t].T P^T
  accumulated over t ([65, 1024]: ones row = softmax denominator).
  qh=0 carries the K/V projection + V transposes; qh=1 is Act-bound and
  carries qh=0's epilogue. Epilogue: reciprocal overwrites the denominator
  row (one row-wide DVE op per half), PE-transpose [65, 128] blocks, then
  per-block tensor_scalar multiplies with the transposed 1/den column.
"""

import sys

for _p in ("/opt/trn_rl_repo",):
    if _p not in sys.path:
        sys.path.insert(0, _p)

from contextlib import ExitStack

import numpy as np
import ml_dtypes

import concourse.bass as bass
import concourse.tile as tile
from concourse import mybir
from concourse.bass_utils import run_bass_kernel_spmd
from concourse.masks import make_identity

BF16 = mybir.dt.bfloat16
F32 = mybir.dt.float32

B, S, D, H = 4, 4096, 1024, 64
NCORES = 8
SQ = S // 2          # queries per core
DC = D // 128        # d contraction chunks
ST = S // 128        # key tiles
QW = 1024            # query width per pass (PSUM budget)
NQH = SQ // QW

_CACHE = {}


def _build_nc(reps=1):
    nc = bass.Bass("TRN2", target_bir_lowering=False, debug=False,
                   num_devices=NCORES)
    xT_d = nc.dram_tensor("xT", [D, S], BF16, kind="ExternalInput")
    wvk_d = nc.dram_tensor("wvk", [D, 128], BF16, kind="ExternalInput")
    wkv_d = nc.dram_tensor("wkv", [D, 128], BF16, kind="ExternalInput")
    wq_d = nc.dram_tensor("wq", [D, 128], BF16, kind="ExternalInput")
    out_d = nc.dram_tensor("out", [SQ, H], F32, kind="ExternalOutput")

    with tile.TileContext(nc) as tc, ExitStack() as ctx:
        _emit(ctx, tc, nc, xT_d.ap(), wvk_d.ap(), wkv_d.ap(), wq_d.ap(),
              out_d.ap(), reps=reps)
    _split_matmul_waits(nc)
    return nc


_SPLIT_OPS = ("Matmult", "Activation", "TensorCopy", "TensorScalarPtr",
              "TensorTensor", "TensorReduce", "Reciprocal", "Memset",
              "Ldweights", "TensorScalarAffineSelect", "Iota",
              "CopyPredicated", "StreamTranspose", "DMACopy", "Drain",
              "NoOp")


def _split_matmul_waits(nc):
    """The 64B compute-instruction encodings hold a single sync wait; Tile
    occasionally attaches two. Hoist the extras onto standalone
    EventSemaphore instructions placed just before the instruction in the
    same engine stream (waits are >=-monotone, so waiting earlier is always
    safe)."""
    n = 0
    for f in nc.m.functions:
        for b in f.blocks:
            new_insts = []
            for i in b.instructions:
                si = getattr(i, "sync_info", None)
                if (i.opcode in _SPLIT_OPS and si is not None and si.on_wait
                        and len(si.on_wait) > 1):
                    for w in list(si.on_wait[:-1]):
                        n += 1
                        ev = mybir.InstEventSemaphore(
                            name=f"I-mmwait-{n}",
                            opcode="EventSemaphore",
                            engine=i.engine,
                            ins=[], outs=[],
                            sync_info=mybir.SyncInfo(on_wait=[w],
                                                     on_update=[]),
                        )
                        new_insts.append(ev)
                    i.sync_info = mybir.SyncInfo(on_wait=[si.on_wait[-1]],
                                                 on_update=si.on_update)
                new_insts.append(i)
            b.instructions = new_insts


def _khalf(t):
    """Partition half holding K^T for key tile t (groups alternate)."""
    return slice(H, 128) if (t // 4) % 2 == 0 else slice(0, H)


def _emit(ctx, tc, nc, xT, wvk, wkv, wq, out, reps=1):
    P = 128
    NSL = 8              # x arrives in 8 seq slices of 512 positions
    SL = S // NSL
    Exp = mybir.ActivationFunctionType.Exp

    xt_pool = ctx.enter_context(tc.tile_pool(name="xt", bufs=1))
    w_pool = ctx.enter_context(tc.tile_pool(name="w", bufs=1))
    vt_pool = ctx.enter_context(tc.tile_pool(name="vt", bufs=2))
    pt_pool = ctx.enter_context(tc.tile_pool(name="pt", bufs=4))
    epi_pool = ctx.enter_context(tc.tile_pool(name="epi", bufs=2))

    wq_sb = w_pool.tile([P, DC, P], BF16, tag="wq")
    wvk_sb = w_pool.tile([P, DC, P], BF16, tag="wvk")
    wkv_sb = w_pool.tile([P, DC, P], BF16, tag="wkv")
    xt = [xt_pool.tile([P, DC, SL], BF16, tag=f"xt{sl}", name=f"xts{sl}")
          for sl in range(NSL)]

    def load_x():
        # DMA order = ramp critical path: wvk feeds the first K/V chunk,
        # x[0:256] feeds it + the first Q chunk, then wq / wkv before
        # their first consumers, then the remaining pieces and slices.
        nc.sync.dma_start(wvk_sb, wvk.rearrange("(o p) h -> p o h", p=P))
        pieces = [(q4 // 2, q4 * 256, (q4 + 1) * 256) for q4 in range(4)]
        pieces += [(sl, sl * SL, (sl + 1) * SL) for sl in range(2, NSL)]
        for i, (sl, c0, c1) in enumerate(pieces):
            nc.sync.dma_start(
                xt[sl][:, :, c0 - sl * SL:c1 - sl * SL],
                xT[:, c0:c1].rearrange("(o p) s -> p o s", p=P))
            if i == 0:
                nc.sync.dma_start(wq_sb,
                                  wq.rearrange("(o p) h -> p o h", p=P))
            elif i == 1:
                nc.sync.dma_start(wkv_sb,
                                  wkv.rearrange("(o p) h -> p o h", p=P))

    def xs(dc, c0, c1):  # x^T[dc*128:(dc+1)*128, c0:c1] from the slice tiles
        sl = c0 // SL
        assert c1 <= (sl + 1) * SL
        return xt[sl][:, dc, c0 - sl * SL:c1 - sl * SL]

    ident = w_pool.tile([H + 1, H + 1], F32, tag="ident")
    make_identity(nc, ident)

    # persistent SBUF state: K^T on partitions 64..127 (even groups) or
    # 0..63 (odd groups); Q^T duplicated on both halves; V_aug with keys
    # on partitions
    kt_sb = w_pool.tile([P, S], BF16, tag="kt")
    qt_sb = w_pool.tile([P, SQ], BF16, tag="qt")
    v_sb = w_pool.tile([P, ST, H + 1], BF16, tag="v")
    nc.vector.memset(v_sb[:, :, H:H + 1], 1.0)

    ps_s = ctx.enter_context(tc.tile_pool(name="pss", bufs=2, space="PSUM"))
    ps_o = ctx.enter_context(tc.tile_pool(name="pso", bufs=1, space="PSUM"))
    ps_a = ctx.enter_context(tc.tile_pool(name="psa", bufs=2, space="PSUM"))

    def q_proj(c0, c1):
        # Q cols [c0, c1) -> qt_sb[:, c0:c1], duplicated on both halves by
        # the widened [Wq|Wq] weights (same stream cost as one half)
        ps = ps_a.tile([P, 512], F32, tag="kvps")
        w = c1 - c0
        for dc in range(DC):
            nc.tensor.matmul(ps[:, 0:w], lhsT=wq_sb[:, dc, :],
                             rhs=xs(dc, c0, c1),
                             start=(dc == 0), stop=(dc == DC - 1))
        nc.vector.tensor_copy(out=qt_sb[:, c0:c1], in_=ps[:, 0:w])

    def kv_proj_mm(g, c0, c1, dc0, dc1, kv=None):
        # packed projection for x cols [c0, c1) of group g: [Wv|Wk] for
        # even groups, [Wk|Wv] for odd, so K's partition half alternates
        if kv is None:
            kv = ps_a.tile([P, 512], F32, tag="kvps")
        w_sb = wvk_sb if g % 2 == 0 else wkv_sb
        w = c1 - c0
        for dc in range(dc0, dc1):
            nc.tensor.matmul(kv[:, 0:w], lhsT=w_sb[:, dc, :],
                             rhs=xs(dc, c0, c1),
                             start=(dc == 0), stop=(dc == DC - 1),
                             skip_group_check=True)
        return kv

    def kv_copies(g, c0, c1, kv):
        w = c1 - c0
        ksl = _khalf(4 * g)
        vsl = slice(0, H) if g % 2 == 0 else slice(H, P)
        nc.vector.tensor_copy(out=kt_sb[ksl, c0:c1], in_=kv[ksl, 0:w])
        vt = vt_pool.tile([H, 512], F32, tag="vt")
        nc.vector.tensor_copy(out=vt[:, 0:w], in_=kv[vsl, 0:w])
        return vt

    def v_trans(t0, vt, w=512):
        # transpose the group's V^T [64, w] into V_aug [keys, 64] tiles
        tr = ps_a.tile([P, 512], F32, tag="kvps")
        for j in range(w // P):
            nc.tensor.transpose(tr[:, j * H:(j + 1) * H],
                                vt[:, j * P:(j + 1) * P], ident[0:H, 0:H])
        nc.vector.tensor_copy(
            out=v_sb[:, t0:t0 + w // P, 0:H],
            in_=tr[:, 0:(w // P) * H].rearrange("p (t h) -> p t h", h=H))

    def s_mm(s_ps, qh, t, qc):
        ksl = _khalf(t)
        nc.tensor.matmul(s_ps[:, qc * 512:(qc + 1) * 512],
                         lhsT=kt_sb[ksl, t * P:(t + 1) * P],
                         rhs=qt_sb[ksl,
                                   qh * QW + qc * 512:qh * QW + (qc + 1) * 512],
                         start=True, stop=True)

    def s_pair(qh, ta, tb, interleave=True, split_b=False):
        # ta is from an even group (K on rows 64..127), tb from an odd
        # group (rows 0..63): interleaved emission lets each qc pair run
        # concurrently on disjoint PE row groups. interleave=False keeps
        # ta's matmuls first so a ramp-stalled tb can't block ta's exp.
        psa = ps_s.tile([P, QW], F32, tag="sps", name="psa")
        psb = ps_s.tile([P, QW], F32, tag="sps", name="psb")
        if interleave:
            s_mm(psa, qh, ta, 0)
            s_mm(psb, qh, tb, 0)
            s_mm(psa, qh, ta, 1)
            s_mm(psb, qh, tb, 1)
        else:
            s_mm(psa, qh, ta, 0)
            s_mm(psa, qh, ta, 1)
            s_mm(psb, qh, tb, 0)
            s_mm(psb, qh, tb, 1)
        pta = pt_pool.tile([P, QW], BF16, tag="pt", name="pta")
        nc.scalar.activation(pta, psa, Exp)
        ptb = pt_pool.tile([P, QW], BF16, tag="pt", name="ptb")
        if not split_b:
            nc.scalar.activation(ptb, psb, Exp)
        else:
            nc.scalar.activation(ptb[:, 0:512], psb[:, 0:512], Exp)
            nc.scalar.activation(ptb[:, 512:1024], psb[:, 512:1024], Exp)
        return pta, ptb

    def av(o_ps, t, pt, qcs=(0, 1), first=False, last=False):
        for qc in qcs:
            nc.tensor.matmul(
                o_ps[:, qc * 512:(qc + 1) * 512],
                lhsT=v_sb[:, t, :],
                rhs=pt[:, qc * 512:(qc + 1) * 512],
                start=first, stop=last)

    def epi_start(o_ps, half=None):
        # stage O^T to SBUF (PE can't read PSUM); the denominator row is
        # replaced by its reciprocal (one row-wide DVE op per half), so the
        # transpose carries 1/den along and no per-block recip is needed
        ot_sb = epi_pool.tile([H + 1, QW], F32, tag="ot")
        halves = (0, 1) if half is None else (half,)
        for hf in halves:
            sl = slice(hf * 512, (hf + 1) * 512)
            nc.vector.tensor_copy(out=ot_sb[0:H, sl], in_=o_ps[0:H, sl])
            nc.vector.reciprocal(ot_sb[H:H + 1, sl], o_ps[H:H + 1, sl])
        return ot_sb

    def epi_more(o_ps, ot_sb, half):
        sl = slice(half * 512, (half + 1) * 512)
        nc.vector.tensor_copy(out=ot_sb[0:H, sl], in_=o_ps[0:H, sl])
        nc.vector.reciprocal(ot_sb[H:H + 1, sl], o_ps[H:H + 1, sl])

    def epi_quad(qh, ot_sb, stage, quad, dma=True):
        # 4 PE transposes [65, 128] -> [128, 65] (col 64 = 1/den), then
        # per-block tensor_scalar multiplies with the PSUM scalar column
        tr = ps_a.tile([P, 512], F32, tag="kvps")
        for j in range(4):
            qt = quad * 4 + j
            nc.tensor.transpose(tr[:, j * (H + 1):(j + 1) * (H + 1)],
                                ot_sb[:, qt * P:(qt + 1) * P], ident)
        for j in range(4):
            c = j * (H + 1)
            nc.vector.tensor_scalar_mul(stage[:, quad * 4 + j, :],
                                        tr[:, c:c + H],
                                        tr[:, c + H:c + H + 1])
        if dma:
            nc.sync.dma_start(
                out[qh * QW + quad * 512:qh * QW + (quad + 1) * 512, :]
                .rearrange("(o p) h -> p o h", p=P),
                stage[:, quad * 4:quad * 4 + 4, :])

    def stage_tile(nm):
        return epi_pool.tile([P, QW // P, H], F32, tag="stage", name=nm)

    warm_sb = w_pool.tile([P, 1], F32, tag="warm")
    warm_out = pt_pool.tile([P, 1], BF16, tag="warmo")
    nc.gpsimd.memset(warm_sb, 0.0)

    def warmup_pe():
        # back-to-back PE work so the tensor engine leaves its low/mid
        # p-state before the first projection, and a dummy 1-col exp so the
        # Act table load happens in the DMA ramp
        nc.scalar.activation(warm_out, warm_sb, Exp)
        for _ in range(3):
            wps = ps_a.tile([P, 512], F32, tag="kvps")
            for r in range(7):
                nc.tensor.transpose(
                    wps[0:H + 1, r * (H + 1):(r + 1) * (H + 1)],
                    ident, ident)

    # pair ordering within each pass: ta from even group, tb from odd
    PAIRS = [(8 * m + j, 8 * m + 4 + j) for m in range(ST // 8)
             for j in range(4)]

    def body(with_dma=True):
        if with_dma:
            load_x()
        warmup_pe()
        # --- ramp: Q chunks + kv groups 0 and 1 in pieces ---------------
        q_proj(0, 256)
        kv0 = kv_proj_mm(0, 0, 128, 0, DC)
        vt0a = kv_copies(0, 0, 128, kv0)
        q_proj(256, 512)
        kv0b = kv_proj_mm(0, 128, 512, 0, DC)
        vt0b = kv_copies(0, 128, 512, kv0b)
        q_proj(512, 1024)
        kv1 = kv_proj_mm(1, 512, 1024, 0, DC)
        v_trans(0, vt0a, w=128)
        vt1 = kv_copies(1, 512, 1024, kv1)
        v_trans(1, vt0b, w=384)
        v_trans(4, vt1)

        # --- qh=0 main loop: carries kv projection + V transposes -------
        # during pair-block m (8 tiles of groups 2m, 2m+1), group 2m+2 is
        # projected over steps j=0,1 and group 2m+3 over j=2,3
        o_ps0 = ps_o.tile([H + 1, QW], F32, tag="ops")
        vts = {}
        prev = []
        kv = None
        for i, (ta, tb) in enumerate(PAIRS):
            m, j = divmod(i, 4)
            pta, ptb = s_pair(0, ta, tb)
            g = 2 * m + 2 + j // 2   # group under projection this step
            if g < ST // 4:
                dc0 = (j % 2) * 4
                kv = kv_proj_mm(g, 512 * g, 512 * (g + 1), dc0, dc0 + 4,
                                None if dc0 == 0 else kv)
            for (t, pt) in prev:
                av(o_ps0, t, pt, first=(t == 0))
            prev = [(ta, pta), (tb, ptb)]
            if g < ST // 4 and j % 2 == 1:
                vts[g] = kv_copies(g, 512 * g, 512 * (g + 1), kv)
            if j == 3:
                for gd in sorted(vts):
                    v_trans(4 * gd, vts.pop(gd))
            if i in (0, 2):          # qh=1's Q projection, 512-col ranges
                c0 = 1024 + 512 * (i // 2)
                q_proj(c0, c0 + 512)
        for (t, pt) in prev:
            av(o_ps0, t, pt, first=(t == 0), last=(t == ST - 1))

        # --- qh=1 loop: Act-bound; qh=0's epilogue rides the slack ------
        o_ps1 = ps_o.tile([H + 1, QW], F32, tag="ops")
        stage0 = stage_tile("stg0")
        epi0 = None
        prev = []
        for i, (ta, tb) in enumerate(PAIRS):
            last = i == len(PAIRS) - 1
            pta, ptb = s_pair(1, ta, tb, split_b=last)
            if i == 0:
                epi0 = epi_start(o_ps0)
            for (t, pt) in prev:
                av(o_ps1, t, pt, first=(t == 0))
            if i in (1, 3):
                epi_quad(0, epi0, stage0, (i - 1) // 2)
            prev = [(ta, pta), (tb, ptb)]
        # tail: tile 31's exp was split per half; AV + epilogue pipeline
        (ta, pta), (tb, ptb) = prev
        av(o_ps1, ta, pta)
        av(o_ps1, tb, ptb, qcs=(0,), last=True)
        ot1 = epi_start(o_ps1, half=0)
        av(o_ps1, tb, ptb, qcs=(1,), last=True)
        stage1 = stage_tile("stg1")
        epi_quad(1, ot1, stage1, 0)
        epi_more(o_ps1, ot1, half=1)
        epi_quad(1, ot1, stage1, 1)

    import os
    dma_once = bool(int(os.environ.get("KBENCH_DMA_ONCE", "0")))
    if reps == 1:
        body()
    else:
        if dma_once:
            load_x()
        with tc.For_i(0, reps):
            body(with_dma=not dma_once)


def make_in_maps(x, Wk, Wq, Wv):
    x = np.asarray(x, dtype=np.float32)
    bf = ml_dtypes.bfloat16
    wk = np.asarray(Wk, np.float32)
    wv = np.asarray(Wv, np.float32)
    wvk = np.concatenate([wv, wk], axis=1).astype(bf)
    wkv = np.concatenate([wk, wv], axis=1).astype(bf)
    # fold the 1/sqrt(HEAD_SIZE) score scale into Wq (exact power of 2);
    # widen to [Wq|Wq] so the projection lands Q on both partition halves
    wqs = np.asarray(Wq, dtype=np.float32) * 0.125
    wq = np.concatenate([wqs, wqs], axis=1).astype(bf)

    in_maps = []
    for c in range(NCORES):
        b, qh = divmod(c, 2)
        xb = x[b]
        if qh:
            xb = np.concatenate([xb[SQ:], xb[:SQ]], axis=0)
        xT = np.ascontiguousarray(xb.T).astype(bf)
        in_maps.append({"xT": xT, "wvk": wvk, "wkv": wkv, "wq": wq})
    return in_maps


def _get_nc():
    if "nc" not in _CACHE:
        _CACHE["nc"] = _build_nc()
    return _CACHE["nc"]


def kernel(x, Wk, Wq, Wv, _trace=False):
    in_maps = make_in_maps(x, Wk, Wq, Wv)
    nc = _get_nc()
    res = run_bass_kernel_spmd(nc, in_maps, core_ids=list(range(NCORES)),
                               trace=_trace)
    _CACHE["last_result"] = res

    out = np.empty((B, S, H), np.float32)
    for c in range(NCORES):
        b, qh = divmod(c, 2)
        out[b, qh * SQ:(qh + 1) * SQ, :] = res.results[c]["out"]
    return out
